# revision 3
# baseline (speedup 1.0000x reference)
"""Sparse (block-causal) GQA attention on 8 Trainium2 NeuronCores.

Problem shapes (hardcoded):
  q: [2048, 4096] f32 (32 heads x 128), k/v: [2048, 1024] f32 (8 kv heads x 128),
  block_mask: [2048, 2048] bool, block-causal at 128 granularity.

Sharding: tensor-parallel over heads. Core c processes q-heads 4c..4c+3 and
kv-head c (the GQA group structure maps exactly onto 8 cores), i.e. contiguous
column slices q[:, 512c:512(c+1)], k[:, 128c:128(c+1)], v[:, 128c:128(c+1)].
The same SPMD program runs on every core; output is the column concat.

Device algorithm per core (bf16 matmuls, f32 accumulation):
  - Q^T (per head) and K^T via PE transposes (f32 in, bf16 out via DVE copy).
  - For each q-chunk of 512 and head: S^T[k, q] = K_kb^T.T @ Q^T chunks in PSUM,
    only for kv-blocks allowed by the block-causal mask (partial-width matmuls
    at the diagonal); exp via ACT (scale folded in) -> P^T bf16 in SBUF;
    o^T[d, q] += V_kb @ P^T and l[q] += ones @ P^T accumulated in PSUM over kb;
    then o = transpose(o^T * (1/l)) and DMA out.
  - Softmax without max-subtraction: scores ~ N(0,1) after 1/sqrt(128) scaling,
    so exp is well within f32 range.
"""

import math

import numpy as np

S = 2048
D = 128
H_LOCAL = 4  # q heads per core
N_CORES = 8
NQB = S // D  # 16 q/k blocks of 128
NQC = 4  # q chunks of 512
QCW = 512  # q chunk width
MASK_BLOCK = 128
SCALE = float(1.0 / np.float32(np.sqrt(np.float32(128.0))))

_cache = {}


def _build_nc():
    import concourse.tile as tile
    from concourse import bacc, mybir
    from concourse.masks import make_identity

    f32 = mybir.dt.float32
    bf16 = mybir.dt.bfloat16

    nc = bacc.Bacc("TRN2", target_bir_lowering=False, debug=False, num_devices=N_CORES)
    q_d = nc.dram_tensor("q", [S, H_LOCAL * D], f32, kind="ExternalInput").ap()
    k_d = nc.dram_tensor("k", [S, D], f32, kind="ExternalInput").ap()
    v_d = nc.dram_tensor("v", [S, D], f32, kind="ExternalInput").ap()
    o_d = nc.dram_tensor("o", [S, H_LOCAL * D], f32, kind="ExternalOutput").ap()

    with tile.TileContext(nc) as tc:
        with (
            tc.tile_pool(name="const", bufs=1) as const_pool,
            tc.tile_pool(name="raw", bufs=1) as raw_pool,
            tc.tile_pool(name="bft", bufs=1) as bft_pool,
            tc.tile_pool(name="pt", bufs=3) as pt_pool,
            tc.tile_pool(name="sm", bufs=2) as sm_pool,
            tc.tile_pool(name="stage", bufs=2) as stage_pool,
            tc.tile_pool(name="st_ps", bufs=2, space="PSUM") as st_ps,
            tc.tile_pool(name="ot_ps", bufs=2, space="PSUM") as ot_ps,
            tc.tile_pool(name="l_ps", bufs=1, space="PSUM") as l_ps,
            tc.tile_pool(name="tr_ps", bufs=1, space="PSUM") as tr_ps,
        ):
            ident = const_pool.tile([128, 128], f32)
            make_identity(nc, ident[:])
            ones = const_pool.tile([128, 128], bf16)
            nc.vector.memset(ones[:], 1.0)

            # ---- load K, V; build K^T (bf16) and V (bf16, k-major tiles) ----
            kraw = raw_pool.tile([128, NQB, D], f32, tag="kraw")
            nc.sync.dma_start(out=kraw[:], in_=k_d.rearrange("(b p) d -> p b d", p=128))
            vraw = raw_pool.tile([128, NQB, D], f32, tag="vraw")
            nc.sync.dma_start(out=vraw[:], in_=v_d.rearrange("(b p) d -> p b d", p=128))
            qraw = raw_pool.tile([128, NQB, H_LOCAL * D], f32, tag="qraw")
            nc.sync.dma_start(out=qraw[:], in_=q_d.rearrange("(b p) c -> p b c", p=128))

            v16 = bft_pool.tile([128, NQB, D], bf16, tag="v16")
            nc.vector.tensor_copy(v16[:], vraw[:])

            kt = bft_pool.tile([128, NQB, D], bf16, tag="kt")  # [d, kb, k]
            # startup transposes rotate across the three 1-bank psum pools so
            # the PSUM->SBUF copies overlap the next round of PE transposes
            def _startup_psum(i):
                # rotate across the 1-bank pools, reusing each pool's main tag
                # so no extra PSUM slots are allocated for startup
                pool, tag = ((tr_ps, "boot"), (l_ps, "lt"), (ot_ps, "ot"))[i % 3]
                return pool.tile([128, 4, 128], f32, tag=tag, name=f"boot{i}")

            r = 0
            for g in range(4):
                tp = _startup_psum(r); r += 1
                for j in range(4):
                    kb = 4 * g + j
                    nc.tensor.transpose(tp[:, j, :], kraw[:, kb, :], ident[:])
                nc.vector.tensor_copy(kt[:, 4 * g : 4 * g + 4, :], tp[:])

            qt = bft_pool.tile([128, H_LOCAL, NQB, D], bf16, tag="qt")  # [d, h, qb, qp]
            for h in range(H_LOCAL):
                for g in range(4):
                    tp = _startup_psum(r); r += 1
                    for j in range(4):
                        qb = 4 * g + j
                        nc.tensor.transpose(
                            tp[:, j, :], qraw[:, qb, h * D : (h + 1) * D], ident[:]
                        )
                    nc.vector.tensor_copy(qt[:, h, 4 * g : 4 * g + 4, :], tp[:])

            # ---- main attention loops ----
            o_r = o_d.rearrange("(qc b p) c -> qc p b c", p=128, b=4)
            for qc in range(NQC):
                stage = stage_pool.tile([128, 4, H_LOCAL * D], f32)
                for h in range(H_LOCAL):
                    n_kb = 4 * qc + 4  # active kv blocks for this q chunk
                    q_lo = qc * QCW
                    ot = ot_ps.tile([128, QCW], f32, tag="ot")
                    lt = l_ps.tile([128, QCW], f32, tag="lt")

                    groups = [list(range(i, i + 2)) for i in range(0, n_kb, 2)]

                    def emit_scores(group):
                        st = st_ps.tile([128, 2, QCW], f32, tag="st")
                        pt = pt_pool.tile([128, 2, QCW], bf16, tag="pt")
                        for gj, kb in enumerate(group):
                            off = max(0, kb * D - q_lo)
                            qb_s = qc * 4 + off // D
                            nc.tensor.matmul(
                                st[:, gj, off:QCW],
                                lhsT=kt[:, kb, :],
                                rhs=qt[:, h, qb_s : qc * 4 + 4, :],
                                start=True,
                                stop=True,
                            )
                        # exp over the whole group tile (unwritten corners are
                        # garbage but their P^T is never consumed below)
                        nc.scalar.activation(pt[:], st[:], mybir.ActivationFunctionType.Exp, scale=SCALE)
                        return pt

                    def emit_pv(group, pt):
                        for gj, kb in enumerate(group):
                            off = max(0, kb * D - q_lo)
                            nc.tensor.matmul(
                                ot[:, off:QCW],
                                lhsT=v16[:, kb, :],
                                rhs=pt[:, gj, off:QCW],
                                start=(kb == 0),
                                stop=(kb == n_kb - 1),
                            )
                            nc.tensor.matmul(
                                lt[:, off:QCW],
                                lhsT=ones[:],
                                rhs=pt[:, gj, off:QCW],
                                start=(kb == 0),
                                stop=(kb == n_kb - 1),
                            )

                    prev = None
                    for group in groups:
                        pt = emit_scores(group)
                        if prev is not None:
                            emit_pv(*prev)
                        prev = (group, pt)
                    emit_pv(*prev)

                    linv = sm_pool.tile([128, QCW], f32, tag="linv")
                    nc.vector.reciprocal(linv[:], lt[:])
                    onorm = sm_pool.tile([128, QCW], f32, tag="onorm")
                    nc.vector.tensor_mul(onorm[:], ot[:], linv[:])

                    tp = tr_ps.tile([128, 4, 128], f32, tag="boot")
                    for j in range(4):
                        nc.tensor.transpose(
                            tp[:, j, :], onorm[:, j * 128 : (j + 1) * 128], ident[:]
                        )
                    nc.vector.tensor_copy(stage[:, :, h * D : (h + 1) * D], tp[:])
                nc.sync.dma_start(out=o_r[qc], in_=stage[:])

    nc.compile()
    return nc


def _get_nc():
    if "nc" not in _cache:
        _cache["nc"] = _build_nc()
    return _cache["nc"]


def _block_causal(mask: np.ndarray) -> bool:
    blk = np.arange(S) // MASK_BLOCK
    return np.array_equal(mask, blk[None, :] <= blk[:, None])


def _numpy_fallback(q, k, v, block_mask):
    qh = q.reshape(S, 32, D).astype(np.float64)
    kh = k.reshape(S, 8, D).astype(np.float64)
    vh = v.reshape(S, 8, D).astype(np.float64)
    kh = np.repeat(kh, 4, axis=1)
    vh = np.repeat(vh, 4, axis=1)
    scores = np.einsum("qhd,khd->hqk", qh, kh) * (1.0 / math.sqrt(D))
    scores = np.where(block_mask[None, :, :], scores, -np.inf)
    scores -= scores.max(axis=-1, keepdims=True)
    p = np.exp(scores)
    p /= p.sum(axis=-1, keepdims=True)
    o = np.einsum("hqk,khd->qhd", p, vh)
    return o.reshape(S, 32 * D).astype(np.float32)


def kernel(q, k, v, block_mask):
    q = np.ascontiguousarray(np.asarray(q, dtype=np.float32))
    k = np.ascontiguousarray(np.asarray(k, dtype=np.float32))
    v = np.ascontiguousarray(np.asarray(v, dtype=np.float32))
    block_mask = np.asarray(block_mask).astype(bool)

    if not _block_causal(block_mask):
        # generic mask: correct (host) fallback; the target workload is the
        # block-causal mask which takes the device path below
        return _numpy_fallback(q, k, v, block_mask)

    from concourse.bass_utils import run_bass_kernel_spmd

    nc = _get_nc()
    in_maps = []
    for c in range(N_CORES):
        in_maps.append(
            {
                "q": np.ascontiguousarray(q[:, c * 512 : (c + 1) * 512]),
                "k": np.ascontiguousarray(k[:, c * 128 : (c + 1) * 128]),
                "v": np.ascontiguousarray(v[:, c * 128 : (c + 1) * 128]),
            }
        )
    res = run_bass_kernel_spmd(nc, in_maps, core_ids=list(range(N_CORES)))
    _cache["last_results"] = res
    return np.concatenate([res.results[c]["o"] for c in range(N_CORES)], axis=1)


# revision 4
# speedup vs baseline: 1.3332x; 1.3332x over previous
"""Sparse (block-causal) GQA attention on 8 Trainium2 NeuronCores.

Problem shapes (hardcoded):
  q: [2048, 4096] f32 (32 heads x 128), k/v: [2048, 1024] f32 (8 kv heads x 128),
  block_mask: [2048, 2048] bool, block-causal at 128 granularity.

Sharding: tensor-parallel over heads. Core c processes q-heads 4c..4c+3 and
kv-head c (the GQA group structure maps exactly onto 8 cores), i.e. contiguous
column slices q[:, 512c:512(c+1)], k[:, 128c:128(c+1)], v[:, 128c:128(c+1)].
The same SPMD program runs on every core; output is the column concat.

Device algorithm per core (bf16 matmuls, f32 accumulation):
  - Q^T (per head) and K^T via PE transposes (f32 in, bf16 out via DVE copy).
  - For each q-chunk of 512 and head: S^T[k, q] = K_kb^T.T @ Q^T chunks in PSUM,
    only for kv-blocks allowed by the block-causal mask (partial-width matmuls
    at the diagonal); exp via ACT (scale folded in) -> P^T bf16 in SBUF;
    o^T[d, q] += V_kb @ P^T and l[q] += ones @ P^T accumulated in PSUM over kb;
    then o = transpose(o^T * (1/l)) and DMA out.  The per-(head, chunk)
    finalization (reciprocal/normalize/output transposes) is software-pipelined
    into the next head's score phase so the PE never stalls on the DVE chain.
  - Softmax without max-subtraction: scores ~ N(0,1) after 1/sqrt(128) scaling,
    so exp is well within f32 range.
"""

import math

import numpy as np

S = 2048
D = 128
H_LOCAL = 4  # q heads per core
N_CORES = 8
NQB = S // D  # 16 q/k blocks of 128
NQC = 4  # q chunks of 512
QCW = 512  # q chunk width
MASK_BLOCK = 128
SCALE = float(1.0 / np.float32(np.sqrt(np.float32(128.0))))

_cache = {}


def _build_nc():
    import concourse.tile as tile
    from concourse import bacc, mybir
    from concourse.masks import make_identity

    f32 = mybir.dt.float32
    bf16 = mybir.dt.bfloat16

    nc = bacc.Bacc("TRN2", target_bir_lowering=False, debug=False, num_devices=N_CORES)
    q_d = nc.dram_tensor("q", [S, H_LOCAL * D], f32, kind="ExternalInput").ap()
    k_d = nc.dram_tensor("k", [S, D], f32, kind="ExternalInput").ap()
    v_d = nc.dram_tensor("v", [S, D], f32, kind="ExternalInput").ap()
    o_d = nc.dram_tensor("o", [S, H_LOCAL * D], f32, kind="ExternalOutput").ap()

    with tile.TileContext(nc) as tc:
        with (
            tc.tile_pool(name="const", bufs=1) as const_pool,
            tc.tile_pool(name="raw", bufs=1) as raw_pool,
            tc.tile_pool(name="bft", bufs=1) as bft_pool,
            tc.tile_pool(name="pt", bufs=3) as pt_pool,
            tc.tile_pool(name="sm", bufs=2) as sm_pool,
            tc.tile_pool(name="stage", bufs=2) as stage_pool,
            tc.tile_pool(name="st_ps", bufs=2, space="PSUM") as st_ps,
            tc.tile_pool(name="ot_ps", bufs=2, space="PSUM") as ot_ps,
            tc.tile_pool(name="l_ps", bufs=1, space="PSUM") as l_ps,
            tc.tile_pool(name="tr_ps", bufs=1, space="PSUM") as tr_ps,
        ):
            ident = const_pool.tile([128, 128], f32)
            make_identity(nc, ident[:])
            ones = const_pool.tile([128, 128], bf16)
            nc.vector.memset(ones[:], 1.0)

            # ---- load K, V first (small), then Q in per-group chunks so the
            # PE transposes can start as soon as each chunk lands ----
            kraw = raw_pool.tile([128, NQB, D], f32, tag="kraw")
            nc.sync.dma_start(out=kraw[:], in_=k_d.rearrange("(b p) d -> p b d", p=128))
            vraw = raw_pool.tile([128, NQB, D], f32, tag="vraw")
            nc.sync.dma_start(out=vraw[:], in_=v_d.rearrange("(b p) d -> p b d", p=128))
            qraw = raw_pool.tile([128, NQB, H_LOCAL * D], f32, tag="qraw")
            q_r = q_d.rearrange("(g b p) c -> g p b c", p=128, b=4)
            for g in range(4):
                nc.sync.dma_start(out=qraw[:, 4 * g : 4 * g + 4, :], in_=q_r[g])

            v16 = bft_pool.tile([128, NQB, D], bf16, tag="v16")
            nc.vector.tensor_copy(v16[:], vraw[:])

            kt = bft_pool.tile([128, NQB, D], bf16, tag="kt")  # [d, kb, k]
            # startup transposes rotate across the three 1-bank psum pools so
            # the PSUM->SBUF copies overlap the next round of PE transposes
            def _startup_psum(i):
                pool, tag = ((tr_ps, "boot"), (l_ps, "lt"), (ot_ps, "ot"))[i % 3]
                return pool.tile([128, 4, 128], f32, tag=tag, name=f"boot{i}")

            r = 0
            for g in range(4):
                tp = _startup_psum(r); r += 1
                for j in range(4):
                    kb = 4 * g + j
                    nc.tensor.transpose(tp[:, j, :], kraw[:, kb, :], ident[:])
                nc.vector.tensor_copy(kt[:, 4 * g : 4 * g + 4, :], tp[:])

            qt = bft_pool.tile([128, H_LOCAL, NQB, D], bf16, tag="qt")  # [d, h, qb, qp]
            for g in range(4):
                for h in range(H_LOCAL):
                    tp = _startup_psum(r); r += 1
                    for j in range(4):
                        qb = 4 * g + j
                        nc.tensor.transpose(
                            tp[:, j, :], qraw[:, qb, h * D : (h + 1) * D], ident[:]
                        )
                    nc.vector.tensor_copy(qt[:, h, 4 * g : 4 * g + 4, :], tp[:])

            # ---- main attention loops ----
            o_r = o_d.rearrange("(qc b p) c -> qc p b c", p=128, b=4)
            state = {"fin": None}

            def flush_fin():
                if state["fin"] is not None:
                    state["fin"]()
                    state["fin"] = None

            for qc in range(NQC):
                stage = stage_pool.tile([128, 4, H_LOCAL * D], f32, name=f"stage{qc}")
                for h in range(H_LOCAL):
                    n_kb = 4 * qc + 4  # active kv blocks for this q chunk
                    q_lo = qc * QCW
                    ot = ot_ps.tile([128, QCW], f32, tag="ot", name=f"ot{qc}_{h}")
                    lt = l_ps.tile([128, QCW], f32, tag="lt", name=f"lt{qc}_{h}")

                    groups = [list(range(i, i + 2)) for i in range(0, n_kb, 2)]

                    def emit_scores(group, h=h, q_lo=q_lo, qc=qc):
                        gid = f"{qc}_{h}_{group[0]}"
                        st = st_ps.tile([128, 2, QCW], f32, tag="st", name=f"st{gid}")
                        pt = pt_pool.tile([128, 2, QCW], bf16, tag="pt", name=f"pt{gid}")
                        for gj, kb in enumerate(group):
                            off = max(0, kb * D - q_lo)
                            qb_s = qc * 4 + off // D
                            nc.tensor.matmul(
                                st[:, gj, off:QCW],
                                lhsT=kt[:, kb, :],
                                rhs=qt[:, h, qb_s : qc * 4 + 4, :],
                                start=True,
                                stop=True,
                            )
                        # exp over the written region of the group tile (the
                        # triangle corners past each kb's own offset hold
                        # garbage, but their P^T is never consumed below)
                        om = max(0, group[0] * D - q_lo)
                        nc.scalar.activation(
                            pt[:, :, om:QCW],
                            st[:, :, om:QCW],
                            mybir.ActivationFunctionType.Exp,
                            scale=SCALE,
                        )
                        return pt

                    def emit_pv(group, pt, h=h, q_lo=q_lo, n_kb=n_kb, ot=ot, lt=lt):
                        for gj, kb in enumerate(group):
                            off = max(0, kb * D - q_lo)
                            nc.tensor.matmul(
                                ot[:, off:QCW],
                                lhsT=v16[:, kb, :],
                                rhs=pt[:, gj, off:QCW],
                                start=(kb == 0),
                                stop=(kb == n_kb - 1),
                            )
                            nc.tensor.matmul(
                                lt[:, off:QCW],
                                lhsT=ones[:],
                                rhs=pt[:, gj, off:QCW],
                                start=(kb == 0),
                                stop=(kb == n_kb - 1),
                            )

                    prev = None
                    for gi, group in enumerate(groups):
                        pt = emit_scores(group)
                        if gi == 1:
                            # overlap the previous head's finalization (DVE
                            # reciprocal/normalize + PE output transposes)
                            # with this head's score phase
                            flush_fin()
                        if prev is not None:
                            emit_pv(*prev)
                        prev = (group, pt)
                    emit_pv(*prev)

                    def finalize(h=h, qc=qc, ot=ot, lt=lt, stage=stage):
                        fid = f"{qc}_{h}"
                        linv = sm_pool.tile([128, QCW], f32, tag="linv", name=f"li{fid}")
                        nc.vector.reciprocal_approx_fast(linv[:], lt[:])
                        onorm = sm_pool.tile([128, QCW], f32, tag="onorm", name=f"on{fid}")
                        nc.vector.tensor_mul(onorm[:], ot[:], linv[:])
                        tp = tr_ps.tile([128, 4, 128], f32, tag="boot", name=f"tp{fid}")
                        for j in range(4):
                            nc.tensor.transpose(
                                tp[:, j, :], onorm[:, j * 128 : (j + 1) * 128], ident[:]
                            )
                        nc.vector.tensor_copy(stage[:, :, h * D : (h + 1) * D], tp[:])
                        if h == H_LOCAL - 1:
                            nc.sync.dma_start(out=o_r[qc], in_=stage[:])

                    state["fin"] = finalize
            flush_fin()

    nc.compile()
    return nc


def _get_nc():
    if "nc" not in _cache:
        _cache["nc"] = _build_nc()
    return _cache["nc"]


def _block_causal(mask: np.ndarray) -> bool:
    blk = np.arange(S) // MASK_BLOCK
    return np.array_equal(mask, blk[None, :] <= blk[:, None])


def _numpy_fallback(q, k, v, block_mask):
    qh = q.reshape(S, 32, D).astype(np.float64)
    kh = k.reshape(S, 8, D).astype(np.float64)
    vh = v.reshape(S, 8, D).astype(np.float64)
    kh = np.repeat(kh, 4, axis=1)
    vh = np.repeat(vh, 4, axis=1)
    scores = np.einsum("qhd,khd->hqk", qh, kh) * (1.0 / math.sqrt(D))
    scores = np.where(block_mask[None, :, :], scores, -np.inf)
    scores -= scores.max(axis=-1, keepdims=True)
    p = np.exp(scores)
    p /= p.sum(axis=-1, keepdims=True)
    o = np.einsum("hqk,khd->qhd", p, vh)
    return o.reshape(S, 32 * D).astype(np.float32)


def kernel(q, k, v, block_mask):
    q = np.ascontiguousarray(np.asarray(q, dtype=np.float32))
    k = np.ascontiguousarray(np.asarray(k, dtype=np.float32))
    v = np.ascontiguousarray(np.asarray(v, dtype=np.float32))
    block_mask = np.asarray(block_mask).astype(bool)

    if not _block_causal(block_mask):
        # generic mask: correct (host) fallback; the target workload is the
        # block-causal mask which takes the device path below
        return _numpy_fallback(q, k, v, block_mask)

    from concourse.bass_utils import run_bass_kernel_spmd

    nc = _get_nc()
    in_maps = []
    for c in range(N_CORES):
        in_maps.append(
            {
                "q": np.ascontiguousarray(q[:, c * 512 : (c + 1) * 512]),
                "k": np.ascontiguousarray(k[:, c * 128 : (c + 1) * 128]),
                "v": np.ascontiguousarray(v[:, c * 128 : (c + 1) * 128]),
            }
        )
    res = run_bass_kernel_spmd(nc, in_maps, core_ids=list(range(N_CORES)))
    _cache["last_results"] = res
    return np.concatenate([res.results[c]["o"] for c in range(N_CORES)], axis=1)
